# revision 1
# baseline (speedup 1.0000x reference)
"""Trainium2 Bass kernel for nn_DetectionLoss (SSD-style detection loss).

Strategy: data-parallel over batch B=8 -> one image per NeuronCore.
Per core, the dense [O=32, A=16384] IoU matching runs as broadcast
tensor_tensor ops over [128 partitions, n=128 anchors/part, o=32] views.
Matched-value extraction uses the (empirically tie-free) one-hot property
of the positive mask.  Each core returns per-partition partial sums plus
the per-anchor negative-CE plane; the host does the final scalar
reductions and the global hard-negative top-k (exactly mirroring the
reference's global sort semantics).
"""

import numpy as np

import concourse.bacc as bacc
import concourse.bass as bass
import concourse.tile as tile
from concourse import mybir
from concourse.bass_utils import run_bass_kernel_spmd

AF = mybir.AluOpType
ACTF = mybir.ActivationFunctionType
AX = mybir.AxisListType
F32 = mybir.dt.float32
I32 = mybir.dt.int32

B, O, A = 8, 32, 16384
P, N = 128, 128          # A = P * N
NCH = 16                  # anchor chunks along n for pipelining
NC_ = N // NCH

# S_out column map (per-partition partials; host sums over partitions/cores)
COL_NPOS0 = 0            # cols [0, NCH): n_pos per chunk
COL_NNEG = 16
COL_SL = 17
COL_SPOS = 18
COL_WSUM = 19


def _chan(apx, c, nch, n=N):
    # [P, n*nch] raw (n-major, c-minor) -> [P, n] plane of channel c
    return apx.rearrange("p (n c) -> p c n", c=nch)[:, c : c + 1, :].squeeze(1)


def _build():
    nc = bacc.Bacc("TRN2", target_bir_lowering=False)
    a_d = nc.dram_tensor("a_raw", [P, 4 * N], F32, kind="ExternalInput")
    p_d = nc.dram_tensor("p_raw", [P, 4 * N], F32, kind="ExternalInput")
    c_d = nc.dram_tensor("c_raw", [P, 2 * N], F32, kind="ExternalInput")
    tb_d = nc.dram_tensor("tb_row", [1, 4 * O], F32, kind="ExternalInput")
    tc_d = nc.dram_tensor("tc_row", [1, O], I32, kind="ExternalInput")
    S_d = nc.dram_tensor("S_out", [P, 24], F32, kind="ExternalOutput")
    ng_d = nc.dram_tensor("negce_out", [P, N], F32, kind="ExternalOutput")

    with tile.TileContext(nc) as tc:
        with (
            tc.tile_pool(name="pl", bufs=1) as pl,
            tc.tile_pool(name="pp", bufs=5) as pp,
        ):
            # ---------------- loads ----------------
            a_sb = pl.tile([P, 4 * N], F32, name="a_sb")
            nc.sync.dma_start(out=a_sb, in_=a_d[:, :])
            p_sb = pl.tile([P, 4 * N], F32, name="p_sb")
            nc.sync.dma_start(out=p_sb, in_=p_d[:, :])
            c_sb = pl.tile([P, 2 * N], F32, name="c_sb")
            nc.sync.dma_start(out=c_sb, in_=c_d[:, :])
            tb_sb = pl.tile([1, 4 * O], F32, name="tb_sb")
            nc.sync.dma_start(out=tb_sb, in_=tb_d[:, :])
            tci_sb = pl.tile([1, O], I32, name="tci_sb")
            nc.sync.dma_start(out=tci_sb, in_=tc_d[:, :])

            S = pl.tile([P, 24], F32, name="S")
            nc.vector.memset(S, 0.0)

            # ---------------- per-object prep on [1, O] rows ----------------
            tcf = pl.tile([1, O], F32, name="tcf")
            nc.vector.tensor_copy(tcf, tci_sb)
            padf = pl.tile([1, O], F32, name="padf")
            nc.vector.tensor_single_scalar(padf, tcf, 0.0, AF.is_lt)
            # row cols (x O): 0 bx1, 1 by1, 2 bx2, 3 by2, 4 bcx, 5 bcy,
            #                 6 lbw, 7 lbh, 8 clsf, 9 areab
            row = pl.tile([1, 10 * O], F32, name="row")
            tmp = pl.tile([1, O], F32, name="tmp")
            FAR = (5.0, 5.0, 6.0, 6.0)  # pad boxes -> far away, IoU = 0
            for c in range(4):
                bcv = _chan(tb_sb, c, 4, n=O)
                rsl = row[:, c * O : (c + 1) * O]
                nc.vector.tensor_scalar(tmp, bcv, -1.0, FAR[c], AF.mult, AF.add)
                nc.vector.scalar_tensor_tensor(rsl, padf, 1.0, tmp, AF.mult, AF.mult)
                nc.vector.tensor_tensor(rsl, rsl, bcv, AF.add)
            for cc, c1, c2 in ((4, 0, 2), (5, 1, 3)):
                nc.vector.tensor_tensor(
                    tmp, row[:, c1 * O : (c1 + 1) * O], row[:, c2 * O : (c2 + 1) * O], AF.add
                )
                nc.vector.tensor_single_scalar(
                    row[:, cc * O : (cc + 1) * O], tmp, 0.5, AF.mult
                )
            nc.vector.tensor_scalar(
                row[:, 8 * O : 9 * O], tcf, 0.0, 1.0, AF.max, AF.min
            )
            # pack cls into the bcx channel: col4 = bcx + 2*clsf (bcx < 1.01)
            nc.vector.scalar_tensor_tensor(
                row[:, 4 * O : 5 * O], row[:, 8 * O : 9 * O], 2.0,
                row[:, 4 * O : 5 * O], AF.mult, AF.add,
            )
            bwh = pl.tile([1, 2 * O], F32, name="bwh")
            nc.vector.tensor_tensor(
                bwh[:, 0:O], row[:, 2 * O : 3 * O], row[:, 0:O], AF.subtract
            )
            nc.vector.tensor_tensor(
                bwh[:, O : 2 * O], row[:, 3 * O : 4 * O], row[:, O : 2 * O], AF.subtract
            )
            nc.scalar.activation(row[:, 6 * O : 8 * O], bwh, ACTF.Ln)
            nc.vector.tensor_tensor(
                row[:, 9 * O : 10 * O], bwh[:, 0:O], bwh[:, O : 2 * O], AF.mult
            )
            # broadcast the whole row across partitions: ones[1,P].T @ row[1,320]
            ones_r = pl.tile([1, P], F32, name="ones_r")
            nc.vector.memset(ones_r, 1.0)
            with tc.tile_pool(name="ps", bufs=1, space="PSUM") as ps:
                bc_ps = ps.tile([P, 10 * O], F32, name="bc_ps")
                nc.tensor.matmul(bc_ps, ones_r, row, start=True, stop=True)
                bc = pl.tile([P, 10 * O], F32, name="bc")
                nc.scalar.copy(bc, bc_ps)

            # ---------------- anchor planes [P, N] ----------------
            cxv = _chan(a_sb, 0, 4)
            cyv = _chan(a_sb, 1, 4)
            wv = _chan(a_sb, 2, 4)
            hv = _chan(a_sb, 3, 4)

            def plane(nm, width=N):
                return pl.tile([P, width], F32, name=nm)

            hwx = plane("hwx")
            nc.vector.tensor_single_scalar(hwx, wv, 0.5, AF.mult)
            hwy = plane("hwy")
            nc.gpsimd.tensor_single_scalar(hwy, hv, 0.5, AF.mult)
            # packed corner planes: a_lo = [ax1|ay1], a_hi = [ax2|ay2]
            a_lo = plane("a_lo", 2 * N)
            a_hi = plane("a_hi", 2 * N)
            nc.vector.tensor_tensor(a_lo[:, 0:N], cxv, hwx, AF.subtract)
            nc.vector.tensor_tensor(a_hi[:, 0:N], cxv, hwx, AF.add)
            nc.gpsimd.tensor_tensor(a_lo[:, N : 2 * N], cyv, hwy, AF.subtract)
            nc.gpsimd.tensor_tensor(a_hi[:, N : 2 * N], cyv, hwy, AF.add)
            area_a = plane("area_a")
            nc.gpsimd.tensor_tensor(area_a, wv, hv, AF.mult)
            wh_view = a_sb.rearrange("p (n c) -> p c n", c=4)[:, 2:4, :]
            logwh = plane("logwh", 2 * N)
            nc.scalar.activation(
                logwh.rearrange("p (c n) -> p c n", n=N), wh_view, ACTF.Ln
            )
            iwh10 = plane("iwh10", 2 * N)
            nc.vector.reciprocal(iwh10.rearrange("p (c n) -> p c n", n=N), wh_view)
            nc.vector.tensor_single_scalar(iwh10, iwh10, 10.0, AF.mult)

            # ---------------- per-anchor class loss planes ----------------
            l0 = _chan(c_sb, 0, 2)
            l1 = _chan(c_sb, 1, 2)
            mx = plane("mx")
            nc.vector.tensor_tensor(mx, l0, l1, AF.max)
            d01 = plane("d01", 2 * N)
            nc.gpsimd.tensor_tensor(d01[:, 0:N], l0, mx, AF.subtract)
            nc.gpsimd.tensor_tensor(d01[:, N : 2 * N], l1, mx, AF.subtract)
            e01 = plane("e01", 2 * N)
            nc.scalar.activation(e01, d01, ACTF.Exp)
            lse = plane("lse")
            nc.gpsimd.tensor_tensor(lse, e01[:, 0:N], e01[:, N : 2 * N], AF.add)
            nc.scalar.activation(lse, lse, ACTF.Ln)
            nc.gpsimd.tensor_tensor(lse, lse, mx, AF.add)
            ce0 = plane("ce0")
            nc.gpsimd.tensor_tensor(ce0, lse, l0, AF.subtract)
            ce1 = plane("ce1")
            nc.gpsimd.tensor_tensor(ce1, lse, l1, AF.subtract)

            best = plane("best")
            thr = plane("thr")
            posa = plane("posa")
            ng = plane("ng")
            ng_u = pl.tile([P, N], mybir.dt.uint32, name="ng_u")
            negce = plane("negce")
            m4 = plane("m4", 4 * N)  # interleaved [p, (n, val)]
            m4r = m4.rearrange("p (n a) -> p a n", a=4)
            m_v1 = m4r[:, 0:1, :].squeeze(1)
            m_bcy = m4r[:, 1:2, :].squeeze(1)
            m_lbw = m4r[:, 2:3, :].squeeze(1)
            m_lbh = m4r[:, 3:4, :].squeeze(1)
            m_bcx = plane("m_bcx")
            m_cls = plane("m_cls")

            # ---------------- pair phase: [P, NC_, O] chunks ----------------
            # Manually software-pipelined: stage A (IoU front) of chunk i+1
            # is emitted before stage B/C tails of chunk i so DVE never
            # stalls on the Pool union/ov chain.
            def pB(q):
                return (
                    bc[:, q * O : (q + 1) * O]
                    .unsqueeze(1)
                    .broadcast_to([P, NC_, O])
                )

            ck = {}

            # static across chunks: sab = area_a[a] + area_b[o], one big op
            sab_full = pl.tile([P, N * O], F32, name="sab_full")
            nc.vector.tensor_tensor(
                sab_full.rearrange("p (n o) -> p n o", o=O),
                area_a.unsqueeze(2).broadcast_to([P, N, O]),
                bc[:, 9 * O : 10 * O].unsqueeze(1).broadcast_to([P, N, O]),
                AF.add,
            )

            def stageA(ci):
                sl = slice(ci * NC_, (ci + 1) * NC_)

                def pA(pln):
                    return pln[:, sl].unsqueeze(2).broadcast_to([P, NC_, O])

                def pA2(pk):
                    # [p, (axis n)] packed plane -> [p, 2, NC_, O] broadcast
                    return (
                        pk.rearrange("p (a n) -> p a n", a=2)[:, :, sl]
                        .unsqueeze(3)
                        .broadcast_to([P, 2, NC_, O])
                    )

                def pB2(q0):
                    # two adjacent bc cols -> [p, 2, NC_, O]
                    return (
                        bc[:, q0 * O : (q0 + 2) * O]
                        .rearrange("p (a o) -> p a o", a=2)
                        .unsqueeze(2)
                        .broadcast_to([P, 2, NC_, O])
                    )

                def pt(nm, mult=1):
                    return pp.tile(
                        [P, mult * NC_ * O], F32, name=f"{nm}{ci}", tag=nm
                    )

                u2 = pt("u2", 2)
                nc.vector.tensor_tensor(
                    u2.rearrange("p (a n o) -> p a n o", a=2, o=O),
                    pA2(a_hi), pB2(2), AF.min,
                )
                v2 = pt("v2", 2)
                nc.vector.tensor_tensor(
                    v2.rearrange("p (a n o) -> p a n o", a=2, o=O),
                    pA2(a_lo), pB2(0), AF.max,
                )
                nc.gpsimd.tensor_tensor(u2, u2, v2, AF.subtract)   # dx|dy raw
                nc.scalar.activation(u2, u2, ACTF.Relu)            # dx|dy (ACT)
                inter = pt("inter")
                nc.gpsimd.tensor_tensor(
                    inter, u2[:, 0 : NC_ * O], u2[:, NC_ * O : 2 * NC_ * O], AF.mult
                )
                union = pt("union")
                nc.gpsimd.tensor_tensor(
                    union, sab_full[:, ci * NC_ * O : (ci + 1) * NC_ * O],
                    inter, AF.subtract,
                )
                ck[ci] = dict(u2=u2, v2=v2, union=union, inter=inter,
                              pt=pt, pA=pA, sl=sl)

            def stageB(ci):
                c = ck[ci]
                rcp = c["pt"]("rcp")
                nc.vector.reciprocal(rcp, c["union"])
                ov = c["pt"]("ov")
                nc.gpsimd.tensor_tensor(ov, c["inter"], rcp, AF.mult)
                c["ov"] = ov

            def stageC(ci):
                c = ck[ci]
                sl, pA = c["sl"], c["pA"]
                ov = c["ov"].rearrange("p (n o) -> p n o", o=O)
                nc.vector.tensor_reduce(best[:, sl], ov, axis=AX.X, op=AF.max)
                nc.vector.tensor_scalar(
                    thr[:, sl], best[:, sl], 1e-6, 0.5, AF.subtract, AF.max
                )
                pos = c["pt"]("pos")
                nc.vector.scalar_tensor_tensor(
                    pos.rearrange("p (n o) -> p n o", o=O), ov, 0.0, pA(thr),
                    AF.add, AF.is_gt,
                    accum_out=S[:, COL_NPOS0 + ci : COL_NPOS0 + ci + 1],
                )
                nc.vector.tensor_single_scalar(posa[:, sl], best[:, sl], 0.5, AF.is_gt)
                # packed extraction: one mult + one reduce over 4 value cols
                mv4 = c["u2"]  # reuse (2*NC_*O) -- need 4*NC_*O; use v2+u2? allocate
                mv4 = c["pt"]("mv4", 4)
                nc.vector.tensor_tensor(
                    mv4.rearrange("p (n a o) -> p n a o", a=4, o=O),
                    pos.rearrange("p (n o) -> p n o", o=O)
                    .unsqueeze(2).broadcast_to([P, NC_, 4, O]),
                    bc[:, 4 * O : 8 * O].rearrange("p (a o) -> p a o", a=4)
                    .unsqueeze(1).broadcast_to([P, NC_, 4, O]),
                    AF.mult,
                )
                nc.vector.tensor_reduce(
                    m4.rearrange("p (n a) -> p n a", a=4)[:, sl],
                    mv4.rearrange("p (n a o) -> p n a o", a=4, o=O),
                    axis=AX.X, op=AF.add,
                )
                del ck[ci]

            sched = []
            for ci in range(NCH):
                sched.append(("A", ci))
            order = []
            emitted_b = emitted_c = 0
            # interleave: A0 A1 B0 A2 B1 C0 A3 B2 C1 B3 C2 C3
            plan = []
            for ci in range(NCH):
                plan.append(("A", ci))
                if ci >= 3:
                    plan.append(("B", ci - 3))
                if ci >= 6:
                    plan.append(("C", ci - 6))
            plan += [("B", ci) for ci in range(NCH - 3, NCH)]
            plan += [("C", ci) for ci in range(NCH - 6, NCH)]
            for st, ci in plan:
                if st == "A":
                    stageA(ci)
                elif st == "B":
                    stageB(ci)
                else:
                    stageC(ci)

            # decode packed extraction: m_cls = m_v1 > 1.5; m_bcx = m_v1 - 2*m_cls
            nc.vector.tensor_single_scalar(m_cls, m_v1, 1.5, AF.is_gt)
            nc.vector.scalar_tensor_tensor(
                m_bcx, m_cls, -2.0, m_v1, AF.mult, AF.add
            )


            nc.vector.tensor_single_scalar(ng, best, 0.5, AF.is_lt)
            nc.vector.tensor_reduce(S[:, COL_NNEG : COL_NNEG + 1], ng, axis=AX.X, op=AF.add)
            nc.gpsimd.tensor_single_scalar(ng_u, best, 0.5, AF.is_lt)
            nc.vector.memset(negce, -1e30)
            nc.vector.copy_predicated(negce, ng_u, ce0)
            nc.sync.dma_start(out=ng_d[:, :], in_=negce)

            # ---------------- box loss ----------------
            g4 = plane("g4", 4 * N)
            nc.vector.tensor_tensor(g4[:, 0:N], m_bcx, cxv, AF.subtract)
            nc.vector.tensor_tensor(g4[:, 0:N], g4[:, 0:N], iwh10[:, 0:N], AF.mult)
            nc.vector.tensor_tensor(g4[:, N : 2 * N], m_bcy, cyv, AF.subtract)
            nc.vector.tensor_tensor(
                g4[:, N : 2 * N], g4[:, N : 2 * N], iwh10[:, N : 2 * N], AF.mult
            )
            nc.vector.tensor_tensor(g4[:, 2 * N : 3 * N], m_lbw, logwh[:, 0:N], AF.subtract)
            nc.vector.tensor_single_scalar(
                g4[:, 2 * N : 3 * N], g4[:, 2 * N : 3 * N], 5.0, AF.mult
            )
            nc.vector.tensor_tensor(
                g4[:, 3 * N : 4 * N], m_lbh, logwh[:, N : 2 * N], AF.subtract
            )
            nc.vector.tensor_single_scalar(
                g4[:, 3 * N : 4 * N], g4[:, 3 * N : 4 * N], 5.0, AF.mult
            )
            d4 = plane("d4", 4 * N)
            for c in range(4):
                eng = nc.vector if c % 2 else nc.gpsimd
                eng.tensor_tensor(
                    d4[:, c * N : (c + 1) * N], _chan(p_sb, c, 4),
                    g4[:, c * N : (c + 1) * N], AF.subtract,
                )
            ad = plane("ad", 4 * N)
            nc.scalar.activation(ad, d4, ACTF.Abs)
            # q = 0.5*ad*ad via ACT Square(scale=sqrt(0.5)); p2 = ad-0.5; m = ad<1
            nc.scalar.activation(d4, ad, ACTF.Square, scale=0.7071067811865476)
            p2 = plane("p2", 4 * N)
            nc.gpsimd.tensor_single_scalar(p2, ad, 0.5, AF.subtract)
            nc.vector.tensor_single_scalar(ad, ad, 1.0, AF.is_lt)
            nc.vector.tensor_tensor(d4, d4, p2, AF.subtract)  # q - p2
            nc.gpsimd.tensor_tensor(d4, ad, d4, AF.mult)      # m*(q-p2)
            nc.vector.tensor_tensor(d4, d4, p2, AF.add)       # smooth_l1
            posa4 = posa.unsqueeze(1).broadcast_to([P, 4, N])
            nc.vector.scalar_tensor_tensor(
                d4.rearrange("p (c n) -> p c n", n=N),
                d4.rearrange("p (c n) -> p c n", n=N),
                1.0, posa4, AF.mult, AF.mult,
                accum_out=S[:, COL_SL : COL_SL + 1],
            )

            # ---------------- positive class loss ----------------
            u = plane("u")
            nc.vector.scalar_tensor_tensor(u, m_cls, 4.0, ce1, AF.mult, AF.mult)
            v2 = plane("v2")
            nc.vector.scalar_tensor_tensor(v2, m_cls, 1.0, ce0, AF.subtract, AF.mult)
            nc.vector.tensor_tensor(u, u, v2, AF.subtract)
            nc.vector.scalar_tensor_tensor(
                u, u, 1.0, posa, AF.mult, AF.mult,
                accum_out=S[:, COL_SPOS : COL_SPOS + 1],
            )
            wa = plane("wa")
            nc.gpsimd.tensor_scalar(wa, m_cls, 3.0, 1.0, AF.mult, AF.add)
            nc.vector.scalar_tensor_tensor(
                wa, wa, 1.0, posa, AF.mult, AF.mult,
                accum_out=S[:, COL_WSUM : COL_WSUM + 1],
            )

            nc.sync.dma_start(out=S_d[:, :], in_=S)
    nc.compile()
    return nc


_CACHE = {}


def _get_nc():
    if "nc" not in _CACHE:
        _CACHE["nc"] = _build()
    return _CACHE["nc"]


def kernel(pred_boxes, pred_classes, true_boxes, true_classes, anchors):
    nc = _get_nc()
    a_raw = np.ascontiguousarray(anchors.reshape(P, 4 * N).astype(np.float32))
    in_maps = []
    for b in range(B):
        in_maps.append(
            dict(
                a_raw=a_raw,
                p_raw=np.ascontiguousarray(
                    pred_boxes[b].reshape(P, 4 * N).astype(np.float32)
                ),
                c_raw=np.ascontiguousarray(
                    pred_classes[b].reshape(P, 2 * N).astype(np.float32)
                ),
                tb_row=np.ascontiguousarray(
                    true_boxes[b].reshape(1, 4 * O).astype(np.float32)
                ),
                tc_row=np.ascontiguousarray(
                    true_classes[b].reshape(1, O).astype(np.int32)
                ),
            )
        )
    res = run_bass_kernel_spmd(nc, in_maps, core_ids=list(range(B)))
    return _combine(res.results)


def _combine(results):
    npos = 0.0
    nneg = 0.0
    sl_sum = 0.0
    spos = 0.0
    wsum = 0.0
    negs = []
    for r in results:
        Sm = r["S_out"].astype(np.float64)
        npos += Sm[:, COL_NPOS0:NCH].sum()
        nneg += Sm[:, COL_NNEG].sum()
        sl_sum += Sm[:, COL_SL].sum()
        spos += Sm[:, COL_SPOS].sum()
        wsum += Sm[:, COL_WSUM].sum()
        negs.append(r["negce_out"].reshape(-1))
    n_pos = int(round(npos))
    n_neg = int(round(nneg))
    denom = float(max(n_pos, 1))
    box_loss = sl_sum / denom
    k = min(10 * n_pos, n_neg)
    allneg = np.concatenate(negs).astype(np.float64)
    if k > 0:
        topk = np.partition(allneg, len(allneg) - k)[len(allneg) - k :]
        sum_neg = float(topk.sum())
    else:
        sum_neg = 0.0
    cls_loss = 10.0 * (spos + sum_neg) / max(wsum + k, 1e-6) / denom
    total = box_loss + cls_loss
    return (
        np.float32(box_loss),
        np.float32(cls_loss),
        np.float32(total),
    )



# revision 18
# speedup vs baseline: 2.0076x; 2.0076x over previous
"""Trainium2 Bass kernel for nn_DetectionLoss (SSD-style detection loss).

Data-parallel over batch B=8 -> one image per NeuronCore.

v4 design notes (baseline was 102.6us):
- Division-free matching: ov > 0.5  <=>  3*inter > area_a + area_b, so
  objects are ranked per anchor by q = inter - area_b/3 and the winner is
  compared against area_a/3.  Removes the per-pair union/reciprocal chain.
- fp16 front on the DVE (min/sub get the 2x mode; the host pre-negates the
  lo corner planes so min/max fuse into one packed min, and pre-duplicates
  the anchor planes x2 so every operand is packed-contiguous).  relu+mult
  fuse into one scalar_tensor_tensor.
- q and all threshold compares are fp32.  Pool legally runs only
  f32 TensorTensor add/sub/mult and TensorScalar ops, so it gets the q
  subtraction, the q-thr subtraction and the >=0 compare.
- Matched-value extraction on the PE: stream-transpose the one-hot mask
  (32x32 blocks put the object axis on partitions), multiply by a
  block-diagonal fp32 value matrix, stream-transpose the PSUM back.
- Tail computed in two n-halves so it overlaps the chunk pipeline.
"""

import numpy as np

import concourse.bacc as bacc
import concourse.tile as tile
from concourse import mybir
from concourse.bass_utils import run_bass_kernel_spmd

AF = mybir.AluOpType
ACTF = mybir.ActivationFunctionType
AX = mybir.AxisListType
F32 = mybir.dt.float32
F16 = mybir.dt.float16

B, O, A = 8, 32, 16384
P, N = 128, 128          # A = P * N
NCH = 8                   # chunks along n
NC_ = N // NCH            # 16 n per chunk
OC = O * NC_              # 512 pairs per chunk

# S_out accumulator columns (per-partition partials, per n-half; host sums)
COL_NPOS, COL_NNEG, COL_SL, COL_SPOS, COL_WSUM = 0, 1, 2, 3, 4
NCOLS = 5  # x2 halves


def _build():
    nc = bacc.Bacc("TRN2", target_bir_lowering=False)
    af_d = nc.dram_tensor("af16", [P, 8 * N + 4 * O], F16, kind="ExternalInput")
    w_d = nc.dram_tensor("w2", [P, P], F32, kind="ExternalInput")
    p4_d = nc.dram_tensor("p4", [P, 4 * N], F32, kind="ExternalInput")
    pc_d = nc.dram_tensor("pc2", [P, 2 * N], F32, kind="ExternalInput")
    en_d = nc.dram_tensor("enc", [P, 8 * N + O], F32, kind="ExternalInput")
    S_d = nc.dram_tensor("S_out", [P, 2 * NCOLS], F32, kind="ExternalOutput")
    ng_d = nc.dram_tensor("negce_out", [P, N], F32, kind="ExternalOutput")

    with tile.TileContext(nc) as tc:
        with (
            tc.tile_pool(name="pl", bufs=1) as pl,
            tc.tile_pool(name="pp", bufs=6) as pp,
            tc.tile_pool(name="ps", bufs=4, space="PSUM") as ps,
        ):
            # ---------------- loads ----------------
            af = pl.tile([P, 8 * N + 4 * O], F16, name="af")
            nc.sync.dma_start(out=af, in_=af_d[:, :])
            w2 = pl.tile([P, P], F32, name="w2")
            nc.sync.dma_start(out=w2, in_=w_d[:, :])
            pc2 = pl.tile([P, 2 * N], F32, name="pc2")
            nc.sync.dma_start(out=pc2, in_=pc_d[:, :])
            en = pl.tile([P, 8 * N + O], F32, name="en")
            nc.sync.dma_start(out=en, in_=en_d[:, :])
            p4 = pl.tile([P, 4 * N], F32, name="p4")
            nc.sync.dma_start(out=p4, in_=p4_d[:, :])

            S = pl.tile([P, 2 * NCOLS], F32, name="S")
            nc.vector.memset(S, 0.0)

            # views into packed inputs
            # af: a_pair[c=4][n=128][t=2] then b[c=4][o=32]
            apair = af[:, 0 : 8 * N].rearrange("p (c n t) -> p c n t", c=4, t=2)
            b4 = af[:, 8 * N :].rearrange("p (c o) -> p c o", c=4)
            b4v = b4.rearrange("p c (m t) -> p c m t", t=2)
            enc = en[:, 0 : 8 * N].rearrange("p (c n) -> p c n", c=8)
            acxy = enc[:, 0:2, :]
            iw10 = enc[:, 2:4, :]
            l5a = enc[:, 4:6, :]
            A3 = enc[:, 6:7, :].squeeze(1)
            A3p = enc[:, 7:8, :].squeeze(1)
            ab3 = en[:, 8 * N :]

            qb_full = pl.tile([P, N], F32, name="qb_full")
            m4_full = pl.tile([P, NCH * OC], F32, name="m4_full")

            # ---------------- class CE planes (chunk-independent) ---------
            l2v = pc2.rearrange("p (c n) -> p c n", c=2)
            l0 = l2v[:, 0:1, :].squeeze(1)
            l1 = l2v[:, 1:2, :].squeeze(1)
            mx = pl.tile([P, N], F32, name="mx")
            nc.vector.tensor_tensor(mx, l0, l1, AF.max)
            d01 = pl.tile([P, 2 * N], F32, name="d01")
            nc.gpsimd.tensor_tensor(
                d01.rearrange("p (c n) -> p c n", c=2), l2v,
                mx.unsqueeze(1).broadcast_to([P, 2, N]), AF.subtract,
            )
            e01 = pl.tile([P, 2 * N], F32, name="e01")
            nc.scalar.activation(e01, d01, ACTF.Exp)
            lse = pl.tile([P, N], F32, name="lse")
            nc.gpsimd.tensor_tensor(lse, e01[:, 0:N], e01[:, N : 2 * N], AF.add)
            nc.scalar.activation(lse, lse, ACTF.Ln)
            nc.gpsimd.tensor_tensor(lse, lse, mx, AF.add)
            ce0 = pl.tile([P, N], F32, name="ce0")
            nc.gpsimd.tensor_tensor(ce0, lse, l0, AF.subtract)
            ce1 = pl.tile([P, N], F32, name="ce1")
            nc.gpsimd.tensor_tensor(ce1, lse, l1, AF.subtract)

            # ---------------- pair-phase chunks (o-minor) ----------------
            ck = {}

            def stageA(ci):
                sl = slice(ci * NC_, (ci + 1) * NC_)

                def pt(nm, w, dt=F16):
                    return pp.tile([P, w], dt, name=f"{nm}{ci}", tag=nm)

                uv = pt("uv", 4 * OC)
                uvv = uv.rearrange("p (c n m t) -> p c n m t", c=4, m=O // 2, t=2)
                for c in range(4):
                    nc.vector.tensor_tensor(
                        uvv[:, c, :, :, :],
                        apair[:, c, sl, :].unsqueeze(2)
                        .broadcast_to([P, NC_, O // 2, 2]),
                        b4v[:, c, :, :].unsqueeze(1)
                        .broadcast_to([P, NC_, O // 2, 2]),
                        AF.min,
                    )
                d2 = pt("d2", 2 * OC)
                nc.vector.tensor_tensor(
                    d2, uv[:, 0 : 2 * OC], uv[:, 2 * OC : 4 * OC], AF.add
                )
                inter = pt("inter", OC, F32)   # relu(dx)*dy, f16 in -> f32 out
                nc.vector.scalar_tensor_tensor(
                    inter, d2[:, 0:OC], 0.0, d2[:, OC : 2 * OC], AF.max, AF.mult
                )
                q = pt("q", OC, F32)
                nc.gpsimd.tensor_tensor(
                    q.rearrange("p (n o) -> p n o", o=O),
                    inter.rearrange("p (n o) -> p n o", o=O),
                    ab3.unsqueeze(1).broadcast_to([P, NC_, O]), AF.subtract,
                )
                ck[ci] = dict(q=q, pt=pt, sl=sl)

            def stageB(ci):
                c = ck[ci]
                q, pt, sl = c["q"], c["pt"], c["sl"]
                qv = q.rearrange("p (n o) -> p n o", o=O)
                qb = qb_full[:, sl]
                nc.vector.tensor_reduce(qb, qv, axis=AX.X, op=AF.max)
                thr = pt("thr", NC_, F32)
                nc.vector.tensor_tensor(thr, qb, A3p[:, sl], AF.max)
                qm = pt("qm", OC, F32)
                nc.gpsimd.tensor_tensor(
                    qm.rearrange("p (n o) -> p n o", o=O), qv,
                    thr.unsqueeze(2).broadcast_to([P, NC_, O]), AF.subtract,
                )
                pos = pt("pos", OC, F32)
                nc.gpsimd.tensor_single_scalar(pos, qm, 0.0, AF.is_ge)
                c["pos"] = pos

            def stageC(ci):
                c = ck[ci]
                pos, pt = c["pos"], c["pt"]
                posT = pt("posT", OC, F32)
                nc.vector.transpose(posT, pos)
                mm = ps.tile([P, OC], F32, name=f"mm{ci}", tag="mm")
                for j in range(OC // P):
                    blk = slice(j * P, (j + 1) * P)
                    nc.tensor.matmul(
                        mm[:, blk], w2, posT[:, blk], start=True, stop=True
                    )
                mmc = pt("mmc", OC, F32)
                nc.scalar.copy(mmc, mm)
                m4c = m4_full[:, ci * OC : (ci + 1) * OC]
                nc.vector.transpose(m4c, mmc)
                del ck[ci]

            # ---------------- tail, one n-half at a time ----------------
            m4v = m4_full.rearrange("p (n k) -> p k n", k=O)  # [P, 32, 128]
            posa = pl.tile([P, N], F32, name="posa")
            ngm = pl.tile([P, N], F32, name="ngm")
            nt1 = pl.tile([P, N], F32, name="nt1")
            negce = pl.tile([P, N], F32, name="negce")
            m_cls = pl.tile([P, N], F32, name="m_cls")
            g4 = pl.tile([P, 4 * N], F32, name="g4")
            g4v = g4.rearrange("p (c n) -> p c n", c=4)
            d4 = pl.tile([P, 4 * N], F32, name="d4")
            d4v = d4.rearrange("p (c n) -> p c n", c=4)
            ad = pl.tile([P, 4 * N], F32, name="ad")
            adv = ad.rearrange("p (c n) -> p c n", c=4)
            cm = pl.tile([P, 4 * N], F32, name="cm")
            cmv = cm.rearrange("p (c n) -> p c n", c=4)
            u_ = pl.tile([P, 4 * N], F32, name="u_")
            uv_ = u_.rearrange("p (c n) -> p c n", c=4)
            slv = pl.tile([P, 4 * N], F32, name="slv")
            slvv = slv.rearrange("p (c n) -> p c n", c=4)
            t1 = pl.tile([P, N], F32, name="t1")
            t2_ = pl.tile([P, N], F32, name="t2_")
            wa = pl.tile([P, N], F32, name="wa")
            p4v = p4.rearrange("p (c n) -> p c n", c=4)

            def tail_half(h):
                H = N // 2
                hs = slice(h * H, (h + 1) * H)
                off = h * NCOLS

                def SC(col):
                    return S[:, off + col : off + col + 1]

                nc.vector.tensor_tensor(posa[:, hs], qb_full[:, hs], A3[:, hs], AF.is_gt)
                nc.vector.tensor_tensor(ngm[:, hs], qb_full[:, hs], A3[:, hs], AF.is_lt)
                nc.vector.tensor_reduce(SC(COL_NPOS), posa[:, hs], axis=AX.X, op=AF.add)
                nc.vector.tensor_reduce(SC(COL_NNEG), ngm[:, hs], axis=AX.X, op=AF.add)
                # negce = ngm*ce0 + (ngm*1e30 - 1e30)  (exact ce0; -1e30 sentinel)
                nc.gpsimd.tensor_tensor(nt1[:, hs], ce0[:, hs], ngm[:, hs], AF.mult)
                nc.gpsimd.tensor_scalar(
                    negce[:, hs], ngm[:, hs], 1e30, 1e30, AF.mult, AF.subtract
                )
                nc.gpsimd.tensor_tensor(
                    negce[:, hs], negce[:, hs], nt1[:, hs], AF.add
                )
                nc.sync.dma_start(out=ng_d[:, hs], in_=negce[:, hs])

                m_c0 = m4v[:, 0:1, hs].squeeze(1)
                nc.vector.tensor_scalar(m_cls[:, hs], m_c0, 1.5, None, AF.is_gt)
                nc.vector.scalar_tensor_tensor(
                    m_c0, m_cls[:, hs], -2.0, m_c0, AF.mult, AF.add
                )
                nc.gpsimd.tensor_tensor(
                    g4v[:, 0:2, hs], m4v[:, 0:2, hs], acxy[:, :, hs], AF.subtract
                )
                nc.gpsimd.tensor_tensor(
                    g4v[:, 0:2, hs], g4v[:, 0:2, hs], iw10[:, :, hs], AF.mult
                )
                nc.gpsimd.tensor_tensor(
                    g4v[:, 2:4, hs], m4v[:, 2:4, hs], l5a[:, :, hs], AF.subtract
                )
                nc.gpsimd.tensor_tensor(
                    d4v[:, :, hs], p4v[:, :, hs], g4v[:, :, hs], AF.subtract
                )
                nc.scalar.activation(adv[:, :, hs], d4v[:, :, hs], ACTF.Abs)
                nc.vector.tensor_scalar(cmv[:, :, hs], adv[:, :, hs], 1.0, None, AF.min)
                nc.vector.scalar_tensor_tensor(
                    uv_[:, :, hs], cmv[:, :, hs], -0.5, adv[:, :, hs], AF.mult, AF.add
                )
                nc.gpsimd.tensor_tensor(
                    slvv[:, :, hs], cmv[:, :, hs], uv_[:, :, hs], AF.mult
                )
                posa4 = posa[:, hs].unsqueeze(1).broadcast_to([P, 4, N // 2])
                nc.vector.scalar_tensor_tensor(
                    slvv[:, :, hs], slvv[:, :, hs], 1.0, posa4,
                    AF.mult, AF.mult, accum_out=SC(COL_SL),
                )
                nc.vector.scalar_tensor_tensor(
                    t1[:, hs], m_cls[:, hs], 4.0, ce1[:, hs], AF.mult, AF.mult
                )
                nc.vector.scalar_tensor_tensor(
                    t2_[:, hs], m_cls[:, hs], 1.0, ce0[:, hs], AF.subtract, AF.mult
                )
                nc.vector.tensor_tensor(t1[:, hs], t1[:, hs], t2_[:, hs], AF.subtract)
                nc.vector.scalar_tensor_tensor(
                    t1[:, hs], t1[:, hs], 1.0, posa[:, hs], AF.mult, AF.mult,
                    accum_out=SC(COL_SPOS),
                )
                nc.gpsimd.tensor_scalar(
                    wa[:, hs], m_cls[:, hs], 3.0, 1.0, AF.mult, AF.add
                )
                nc.vector.scalar_tensor_tensor(
                    wa[:, hs], wa[:, hs], 1.0, posa[:, hs], AF.mult, AF.mult,
                    accum_out=SC(COL_WSUM),
                )

            # schedule: A leads B by 3, B leads C by 3; tail halves interleave
            plan = []
            for ci in range(NCH):
                plan.append(("A", ci))
                if ci >= 3:
                    plan.append(("B", ci - 3))
                if ci >= 6:
                    plan.append(("C", ci - 6))
            plan += [("B", NCH - 3), ("C", NCH - 6), ("B", NCH - 2),
                     ("C", NCH - 5), ("B", NCH - 1), ("C", NCH - 4),
                     ("C", NCH - 3), ("T", 0), ("C", NCH - 2), ("C", NCH - 1),
                     ("T", 1)]
            for st, ci in plan:
                if st == "A":
                    stageA(ci)
                elif st == "B":
                    stageB(ci)
                elif st == "C":
                    stageC(ci)
                else:
                    tail_half(ci)

            nc.sync.dma_start(out=S_d[:, :], in_=S)
    nc.compile()
    return nc


_CACHE = {}


def _get_nc():
    if "nc" not in _CACHE:
        _CACHE["nc"] = _build()
    return _CACHE["nc"]


def _point_form(c):
    return np.concatenate([c[..., :2] - c[..., 2:] / 2, c[..., :2] + c[..., 2:] / 2], -1)


def _prep_maps(pred_boxes, pred_classes, true_boxes, true_classes, anchors):
    f32, f16 = np.float32, np.float16
    an = np.asarray(anchors, f32)
    pf = _point_form(an)                                   # [A,4] corners
    aw, ah = an[:, 2], an[:, 3]
    A3 = (aw * ah / 3.0).astype(f32)
    A3p = np.nextafter(A3, np.float32(np.inf)).astype(f32)

    # a_pair[c][n][2] (x2-duplicated anchor planes, lo pre-negated), f16
    afix = np.empty((P, 8 * N + 4 * O), f16)
    ap4 = np.empty((P, 4, N, 2), f16)
    ap4[:, 0, :, 0] = ap4[:, 0, :, 1] = pf[:, 2].reshape(P, N).astype(f16)
    ap4[:, 1, :, 0] = ap4[:, 1, :, 1] = pf[:, 3].reshape(P, N).astype(f16)
    ap4[:, 2, :, 0] = ap4[:, 2, :, 1] = (-pf[:, 0]).reshape(P, N).astype(f16)
    ap4[:, 3, :, 0] = ap4[:, 3, :, 1] = (-pf[:, 1]).reshape(P, N).astype(f16)
    afix[:, 0 : 8 * N] = ap4.reshape(P, 8 * N)

    encp = np.empty((P, 8 * N + O), f32)
    encp[:, 0:N] = an[:, 0].reshape(P, N)
    encp[:, N:2 * N] = an[:, 1].reshape(P, N)
    encp[:, 2 * N:3 * N] = (10.0 / aw).reshape(P, N)
    encp[:, 3 * N:4 * N] = (10.0 / ah).reshape(P, N)
    encp[:, 4 * N:5 * N] = (5.0 * np.log(aw)).reshape(P, N)
    encp[:, 5 * N:6 * N] = (5.0 * np.log(ah)).reshape(P, N)
    encp[:, 6 * N:7 * N] = A3.reshape(P, N)
    encp[:, 7 * N:8 * N] = A3p.reshape(P, N)

    FAR = np.array([5.0, 5.0, 6.0, 6.0], f32)
    in_maps = []
    for b in range(B):
        tb = np.asarray(true_boxes[b], f32)
        tc = np.asarray(true_classes[b])
        pad = tc < 0
        tbm = np.where(pad[:, None], FAR[None, :], tb)
        bw = tbm[:, 2] - tbm[:, 0]
        bh = tbm[:, 3] - tbm[:, 1]
        ab3 = (bw * bh / 3.0).astype(f32)

        afb = afix.copy()
        afb[:, 8 * N + 0 * O : 8 * N + 1 * O] = tbm[:, 2].astype(f16)[None, :]
        afb[:, 8 * N + 1 * O : 8 * N + 2 * O] = tbm[:, 3].astype(f16)[None, :]
        afb[:, 8 * N + 2 * O : 8 * N + 3 * O] = (-tbm[:, 0]).astype(f16)[None, :]
        afb[:, 8 * N + 3 * O : 8 * N + 4 * O] = (-tbm[:, 1]).astype(f16)[None, :]

        enb = encp.copy()
        enb[:, 8 * N:] = ab3[None, :]

        cls = np.clip(tc, 0, 1).astype(f32)
        vals = np.stack([
            (tbm[:, 0] + tbm[:, 2]) * 0.5 + 2.0 * cls,
            (tbm[:, 1] + tbm[:, 3]) * 0.5,
            5.0 * np.log(bw), 5.0 * np.log(bh),
        ], -1).astype(f32)                                  # [O,4]
        w2 = np.zeros((P, P), f32)
        for pb in range(4):
            w2[32 * pb:32 * pb + 32, 32 * pb:32 * pb + 4] = vals

        p4 = np.ascontiguousarray(
            np.asarray(pred_boxes[b], f32).reshape(P, N, 4).transpose(0, 2, 1)
            .reshape(P, 4 * N))
        pc = np.ascontiguousarray(
            np.asarray(pred_classes[b], f32).reshape(P, N, 2).transpose(0, 2, 1)
            .reshape(P, 2 * N))
        in_maps.append(dict(af16=afb, w2=w2, p4=p4, pc2=pc, enc=enb))
    return in_maps


def kernel(pred_boxes, pred_classes, true_boxes, true_classes, anchors):
    nc = _get_nc()
    in_maps = _prep_maps(pred_boxes, pred_classes, true_boxes, true_classes, anchors)
    res = run_bass_kernel_spmd(nc, in_maps, core_ids=list(range(B)))
    return _combine(res.results)


def _combine(results):
    npos = nneg = sl_sum = spos = wsum = 0.0
    negs = []
    for r in results:
        Sm = r["S_out"].astype(np.float64)
        for off in (0, NCOLS):
            npos += Sm[:, off + COL_NPOS].sum()
            nneg += Sm[:, off + COL_NNEG].sum()
            sl_sum += Sm[:, off + COL_SL].sum()
            spos += Sm[:, off + COL_SPOS].sum()
            wsum += Sm[:, off + COL_WSUM].sum()
        negs.append(r["negce_out"].reshape(-1))
    n_pos = int(round(npos))
    n_neg = int(round(nneg))
    denom = float(max(n_pos, 1))
    box_loss = sl_sum / denom
    k = min(10 * n_pos, n_neg)
    allneg = np.concatenate(negs).astype(np.float64)
    if k > 0:
        topk = np.partition(allneg, len(allneg) - k)[len(allneg) - k:]
        sum_neg = float(topk.sum())
    else:
        sum_neg = 0.0
    cls_loss = 10.0 * (spos + sum_neg) / max(wsum + k, 1e-6) / denom
    total = box_loss + cls_loss
    return (np.float32(box_loss), np.float32(cls_loss), np.float32(total))
